# revision 7
# baseline (speedup 1.0000x reference)
"""Trainium2 Bass kernel for nn_Loss_39341900431615.

Reference semantics (B,C,H,W = 16,128,128,128; only tensor[0] is read):
    idx = argmax(tensor[0,0].reshape(-1))        # row-major first max
    x0, y0 = idx // W, idx % W
    wgt[j,k] = (x0-j)^2 + (y0-k)^2               # [H,W] = [128,128]
    out[w] = sum_{j,k} wgt[j,k] * tensor[0,j,k,w]  # [W] = [128]

Sharding: j (channel dim of tensor[0]) is split across 8 cores, 16
j-planes each (1 MB/core). Each core redundantly computes the argmax
from a replicated copy of tensor[0,0] and emits a [128] partial; the
host sums the 8 partials.

Key restructure vs the v1 kernel (which serialized argmax -> wgt ->
reduction): the weight factors as
    wgt[p,klo] = q0*1 + q1*jl(p) + q2*k(p,klo) + (jl(p)^2 + k(p,klo)^2)
with q0 = x0'^2+y0^2, q1 = -2*x0', q2 = -2*y0, x0' = x0 - jlo, and
jl(p) = p//8 the core-local j. So the big reduction is FOUR fixed-weight
sums R_i[w] = sum_{p,klo} C_i[p,klo]*st[p,klo,w] that do not depend on
the argmax at all: they run as PE matmuls (stationary C [128,4] f32r,
moving st [128,128] f32r, accumulating PSUM [4,128]) as soon as the
data lands. The argmax chain only has to produce three scalars in time
for a 3-op DVE combine at the very end:
    out[w] = q0*R0 + q1*R1 + q2*R2 + R3.

DMA plan (the v1 bottleneck was a [128 x 1096B] const-blob DMA whose
128 per-partition descriptors took ~2.5us to complete, plus consts
gated the whole chain):
  - no const blob at all: iota/shift/mask ops on GpSimd generate the
    C matrix, flat-index rows, etc. during the DMA shadow. The only
    per-core varying scalar (jlo) rides as one extra column of the map.
  - map+meta [32, 513] f32: 32 contiguous ~2KB descriptors (fast path).
  - tslice [128, 16, 128] f32r in TWO half-DMAs (klo 0..7, 8..15) on
    the ACT ring: first-half matmuls start ~1.7us before the tail of
    the transfer arrives.
  - fp32r matmuls: single-pass fp32 on the PE (tolerance is 2e-2;
    measured error stays ~1e-6 regardless).

Argmax without transposes (order-free because the max is unique in the
reference's random data): gmax = all-reduce max (GpSimd XYZWC), flat =
sum((map==gmax)*flatidx) via one DVE STT with accum_out + a GpSimd
C-axis reduce; x0 = flat>>7, y0 = flat&127 in int32; broadcastless
combine because the three q scalars and the PSUM result rows all live
on partitions 0..3 as [1,128]/[1,1] APs.

Framework facts this code is shaped by (measured on this machine):
  - walrus allows ONE sync wait per compute instruction; Bacc's
    generate_event_semaphores/move_matmul_waits_to_ldweights legalize
    multi-wait instructions, raw bass.Bass does not -> use bacc.Bacc
    and call nc.finalize() before compiling/running.
  - Bacc DCE removes dead instructions WITH their semaphore waits --
    never park a DMA wait on an instruction whose output nobody reads.
  - NRT adds ~11 us of fixed per-execution overhead (entry barrier +
    engine TENSOR_LOADs at ~3.4-4.9us + Tile preamble barrier to
    ~7.2us + full semaphore-space sweep at exit ~3.4us).
"""

import sys

for _p in ("/opt/trn_rl_repo", "/opt/pypackages"):
    if _p not in sys.path:
        sys.path.insert(0, _p)

import numpy as np

import concourse.bass as bass
from concourse import bacc
import concourse.tile as tile
from concourse import mybir
from concourse import bass_isa
from concourse.bass_utils import run_bass_kernel_spmd

B, C, H, W = 16, 128, 128, 128
NCORES = 8
JPER = C // NCORES   # 16 j-planes per core
KLO = 16             # contraction steps per partition (k within block)
KHI = 8              # k blocks per partition dim
MAPP = 32            # partitions of the contiguous map load
MAPF = (H * W) // MAPP  # 512 map elems per partition
NDMA = 2             # tslice split into this many klo-chunks

F32 = mybir.dt.float32
F32R = mybir.dt.float32r
I32 = mybir.dt.int32
AX = mybir.AxisListType
OP = mybir.AluOpType

_CACHE = {}


def _build_bass():
    nc = bacc.Bacc("TRN2", target_bir_lowering=False, debug=False,
                   num_devices=NCORES, enable_partition_id=False)

    # map+meta: cols 0..511 = tensor[0,0] row-major; col 512 row 0 = jlo
    map_d = nc.dram_tensor("map", [MAPP, MAPF + 1], F32, kind="ExternalInput")
    ts_d = nc.dram_tensor("tslice", [128, KLO, W], F32R, kind="ExternalInput")
    outd = nc.dram_tensor("out", [1, W], F32, kind="ExternalOutput")

    with tile.TileContext(nc) as tc:
        with (
            tc.tile_pool(name="main", bufs=1) as pool,
            tc.tile_pool(name="psum", bufs=1, space="PSUM") as psum_pool,
        ):
            mp = pool.tile([MAPP, MAPF + 1], F32)
            st = pool.tile([128, KLO, W], F32R)

            # --- input DMAs (map first: it gates the scalar chain) ---
            nc.sync.dma_start(out=mp[:, :], in_=map_d[:, :])
            kper = KLO // NDMA
            for c in range(NDMA):
                k0 = c * kper
                nc.scalar.dma_start(out=st[:, k0:k0 + kper, :],
                                    in_=ts_d.ap()[:, k0:k0 + kper, :])

            # --- const generation on GpSimd (runs in the DMA shadow) ---
            flatidx = pool.tile([MAPP, MAPF], F32)
            nc.gpsimd.iota(flatidx[:, :], [[1, MAPF]], channel_multiplier=MAPF,
                           allow_small_or_imprecise_dtypes=True)

            itp = pool.tile([128, 1], I32)
            nc.gpsimd.iota(itp[:, :], [[0, 1]], channel_multiplier=1)
            itk = pool.tile([128, KLO], I32)
            nc.gpsimd.iota(itk[:, :], [[1, KLO]], channel_multiplier=0)

            jli = pool.tile([128, 1], I32)
            nc.vector.tensor_scalar(jli, itp, 3, None, op0=OP.arith_shift_right)
            kbi = pool.tile([128, 1], I32)
            nc.vector.tensor_scalar(kbi, itp, 7, None, op0=OP.bitwise_and)
            nc.vector.tensor_scalar(kbi, kbi, 4, None, op0=OP.logical_shift_left)

            jlf = pool.tile([128, 1], F32)
            nc.gpsimd.tensor_copy(jlf, jli)
            kbf = pool.tile([128, 1], F32)
            nc.gpsimd.tensor_copy(kbf, kbi)
            ktf = pool.tile([128, KLO], F32)
            nc.gpsimd.tensor_copy(ktf, itk)
            kvf = pool.tile([128, KLO], F32)
            nc.gpsimd.tensor_scalar(kvf, ktf, kbf[:, 0:1], None, op0=OP.add)

            # C weight matrix [p, klo, 4] = [1, jl, k, jl^2+k^2]
            c32 = pool.tile([128, KLO, 4], F32)
            nc.gpsimd.memset(c32[:, :, 0:1], 1.0)
            nc.gpsimd.tensor_scalar(c32[:, :, 1:2], c32[:, :, 0:1],
                                    jlf[:, 0:1], None, op0=OP.mult)
            nc.gpsimd.tensor_copy(c32[:, :, 2:3], kvf[:, :])
            jjf = pool.tile([128, 1], F32)
            nc.gpsimd.tensor_tensor(jjf, jlf, jlf, op=OP.mult)
            kkf = pool.tile([128, KLO], F32)
            nc.gpsimd.tensor_tensor(kkf, kvf, kvf, op=OP.mult)
            nc.gpsimd.tensor_scalar(c32[:, :, 3:4], kkf[:, :],
                                    jjf[:, 0:1], None, op0=OP.add)
            cw = pool.tile([128, KLO, 4], F32R)
            nc.gpsimd.tensor_copy(cw[:, :, :], c32[:, :, :])

            # --- argmax scalars (gated only by the map DMA) ---
            sm = mp[:, 0:MAPF]
            rmax = pool.tile([MAPP, 1], F32)
            nc.vector.tensor_reduce(rmax, sm, axis=AX.X, op=OP.max)
            gmaxb = pool.tile([MAPP, 1], F32)
            nc.gpsimd.partition_all_reduce(gmaxb[:, 0:1], rmax[:, 0:1],
                                           channels=MAPP,
                                           reduce_op=bass_isa.ReduceOp.max)

            onehot = pool.tile([MAPP, MAPF], F32)
            fa = pool.tile([MAPP, 1], F32)
            nc.vector.scalar_tensor_tensor(
                onehot, in0=sm, scalar=gmaxb[:, 0:1], in1=flatidx,
                op0=OP.is_equal, op1=OP.mult, accum_out=fa[:, 0:1])
            flatb = pool.tile([MAPP, 1], F32)
            nc.gpsimd.partition_all_reduce(flatb[:, 0:1], fa[:, 0:1],
                                           channels=MAPP,
                                           reduce_op=bass_isa.ReduceOp.add)

            flti = pool.tile([1, 1], I32)
            nc.vector.tensor_copy(flti, flatb[0:1, 0:1])
            y0i = pool.tile([1, 1], I32)
            nc.vector.tensor_scalar(y0i, flti, 127, None, op0=OP.bitwise_and)
            x0i = pool.tile([1, 1], I32)
            nc.vector.tensor_scalar(x0i, flti, 7, None,
                                    op0=OP.logical_shift_right)
            y0f = pool.tile([1, 1], F32)
            nc.vector.tensor_copy(y0f, y0i)
            x0f = pool.tile([1, 1], F32)
            nc.vector.tensor_copy(x0f, x0i)
            x0p = pool.tile([1, 1], F32)   # x0' = x0 - jlo
            nc.vector.tensor_tensor(x0p, x0f, mp[0:1, MAPF:MAPF + 1],
                                    op=OP.subtract)

            # q row = [x0'^2 + y0^2, -2*x0', -2*y0, 1]
            qrow = pool.tile([1, 4], F32)
            nc.vector.memset(qrow[:, 3:4], 1.0)
            nc.vector.tensor_scalar(qrow[:, 1:2], x0p, -2.0, None, op0=OP.mult)
            nc.vector.tensor_scalar(qrow[:, 2:3], y0f, -2.0, None, op0=OP.mult)
            xx = pool.tile([1, 1], F32)
            nc.vector.tensor_tensor(xx, x0p, x0p, op=OP.mult)
            nc.vector.scalar_tensor_tensor(
                qrow[:, 0:1], in0=y0f, scalar=y0f[:, 0:1], in1=xx,
                op0=OP.mult, op1=OP.add)

            # qcol [4,1] = qrow.T without touching the busy PE: broadcast the
            # row to partitions 0..3, mask the diagonal, row-reduce.
            qb = pool.tile([4, 4], F32)
            nc.gpsimd.partition_broadcast(qb[:, :], qrow[:, :])
            qdiag = pool.tile([4, 4], F32)
            nc.gpsimd.affine_select(qdiag, qb, [[1, 4]], OP.is_equal, 0.0,
                                    base=0, channel_multiplier=-1)
            qcol = pool.tile([4, 1], F32)
            nc.vector.tensor_reduce(qcol, qdiag, axis=AX.X, op=OP.add)

            # --- main reduction: PSUM[4, w] += C[:,klo,:].T @ st[:,klo,:] ---
            psr = psum_pool.tile([4, W], F32)
            for klo in range(KLO):
                nc.tensor.matmul(psr[:, :], cw[:, klo, :], st[:, klo, :],
                                 start=(klo == 0), stop=(klo == KLO - 1))

            # --- combine: out = qcol.T @ [R0;R1;R2;R3] on the PE ---
            r4 = pool.tile([4, W], F32)
            nc.vector.tensor_copy(r4, psr[:, :])
            outp = psum_pool.tile([1, W], F32)
            nc.tensor.matmul(outp[:, :], qcol[:, :], r4[:, :],
                             start=True, stop=True)
            outv = pool.tile([1, W], F32)
            nc.vector.tensor_copy(outv, outp[:, :])

            nc.sync.dma_start(out=outd[:, :], in_=outv[:, :])

    return nc


def _get_bass():
    if "nc" not in _CACHE:
        nc = _build_bass()
        nc.finalize()
        _CACHE["nc"] = nc
    return _CACHE["nc"]


def _make_in_maps(tensor):
    t0 = np.ascontiguousarray(tensor[0], dtype=np.float32)  # [C,H,W]
    mp0 = t0[0].reshape(MAPP, MAPF)
    in_maps = []
    for c in range(NCORES):
        jlo = c * JPER
        mapx = np.empty((MAPP, MAPF + 1), dtype=np.float32)
        mapx[:, :MAPF] = mp0
        mapx[:, MAPF] = float(jlo)
        in_maps.append({
            "map": mapx,
            "tslice": np.ascontiguousarray(
                t0[jlo:jlo + JPER].reshape(128, KLO, W)),
        })
    return in_maps


def kernel(tensor):
    nc = _get_bass()
    res = run_bass_kernel_spmd(nc, _make_in_maps(tensor),
                               core_ids=list(range(NCORES)))
    partials = np.stack([r["out"].reshape(W) for r in res.results])
    return partials.astype(np.float64).sum(axis=0).astype(np.float32)


# revision 8
# speedup vs baseline: 1.4046x; 1.4046x over previous
"""Trainium2 Bass kernel for nn_Loss_39341900431615.

Reference semantics (B,C,H,W = 16,128,128,128; only tensor[0] is read):
    idx = argmax(tensor[0,0].reshape(-1))        # row-major first max
    x0, y0 = idx // W, idx % W
    wgt[j,k] = (x0-j)^2 + (y0-k)^2               # [H,W] = [128,128]
    out[w] = sum_{j,k} wgt[j,k] * tensor[0,j,k,w]  # [W] = [128]

Sharding: j (channel dim of tensor[0]) is split across 8 cores, 16
j-planes each (1 MB/core). Each core redundantly computes the argmax
from a replicated copy of tensor[0,0] and emits a [128] partial; the
host sums the 8 partials.

Key restructure vs the v1 kernel (which serialized argmax -> wgt ->
reduction): the weight factors as
    wgt[p,klo] = q0*1 + q1*jl(p) + q2*k(p,klo) + (jl(p)^2 + k(p,klo)^2)
with q0 = x0'^2+y0^2, q1 = -2*x0', q2 = -2*y0, x0' = x0 - jlo, and
jl(p) = p//8 the core-local j. So the big reduction is FOUR fixed-weight
sums R_i[w] = sum_{p,klo} C_i[p,klo]*st[p,klo,w] that do not depend on
the argmax at all: they run as PE matmuls (stationary C [128,4] f32r,
moving st [128,128] f32r, accumulating PSUM [4,128]) as soon as the
data lands. The argmax chain only has to produce three scalars in time
for a 3-op DVE combine at the very end:
    out[w] = q0*R0 + q1*R1 + q2*R2 + R3.

DMA plan (the v1 bottleneck was a [128 x 1096B] const-blob DMA whose
128 per-partition descriptors took ~2.5us to complete, plus consts
gated the whole chain):
  - no const blob at all: iota/shift/mask ops on GpSimd generate the
    C matrix, flat-index rows, etc. during the DMA shadow. The only
    per-core varying scalar (jlo) rides as one extra column of the map.
  - map+meta [32, 513] f32: 32 contiguous ~2KB descriptors (fast path).
  - tslice [128, 16, 128] f32r in TWO half-DMAs (klo 0..7, 8..15) on
    the ACT ring: first-half matmuls start ~1.7us before the tail of
    the transfer arrives.
  - fp32r matmuls: single-pass fp32 on the PE (tolerance is 2e-2;
    measured error stays ~1e-6 regardless).

Argmax without transposes (order-free because the max is unique in the
reference's random data): gmax = all-reduce max (GpSimd XYZWC), flat =
sum((map==gmax)*flatidx) via one DVE STT with accum_out + a GpSimd
C-axis reduce; x0 = flat>>7, y0 = flat&127 in int32; broadcastless
combine because the three q scalars and the PSUM result rows all live
on partitions 0..3 as [1,128]/[1,1] APs.

Framework facts this code is shaped by (measured on this machine):
  - walrus allows ONE sync wait per compute instruction; Bacc's
    generate_event_semaphores/move_matmul_waits_to_ldweights legalize
    multi-wait instructions, raw bass.Bass does not -> use bacc.Bacc
    and call nc.finalize() before compiling/running.
  - Bacc DCE removes dead instructions WITH their semaphore waits --
    never park a DMA wait on an instruction whose output nobody reads.
  - NRT adds ~11 us of fixed per-execution overhead (entry barrier +
    engine TENSOR_LOADs at ~3.4-4.9us + Tile preamble barrier to
    ~7.2us + full semaphore-space sweep at exit ~3.4us).
"""

import sys

for _p in ("/opt/trn_rl_repo", "/opt/pypackages"):
    if _p not in sys.path:
        sys.path.insert(0, _p)

import numpy as np

import concourse.bass as bass
from concourse import bacc
import concourse.tile as tile
from concourse import mybir
from concourse import bass_isa
from concourse.bass_utils import run_bass_kernel_spmd

B, C, H, W = 16, 128, 128, 128
NCORES = 8
JPER = C // NCORES   # 16 j-planes per core
KLO = 16             # contraction steps per partition (k within block)
KHI = 8              # k blocks per partition dim
MAPP = 32            # partitions of the contiguous map load
MAPF = (H * W) // MAPP  # 512 map elems per partition
NDMA = 2             # tslice split into this many klo-chunks

F32 = mybir.dt.float32
F32R = mybir.dt.float32r
I32 = mybir.dt.int32
AX = mybir.AxisListType
OP = mybir.AluOpType

_CACHE = {}


def _build_bass():
    nc = bacc.Bacc("TRN2", target_bir_lowering=False, debug=False,
                   num_devices=NCORES, enable_partition_id=False)

    # map+meta: cols 0..511 = tensor[0,0] row-major; col 512 row 0 = jlo
    map_d = nc.dram_tensor("map", [MAPP, MAPF + 1], F32, kind="ExternalInput")
    ts_d = nc.dram_tensor("tslice", [128, KLO, W], F32R, kind="ExternalInput")
    outd = nc.dram_tensor("out", [1, W], F32, kind="ExternalOutput")

    with tile.TileContext(nc) as tc:
        with (
            tc.tile_pool(name="main", bufs=1) as pool,
            tc.tile_pool(name="psum", bufs=1, space="PSUM") as psum_pool,
        ):
            mp = pool.tile([MAPP, MAPF + 1], F32)
            st = pool.tile([128, KLO, W], F32R)

            # --- input DMAs (map first: it gates the scalar chain) ---
            nc.sync.dma_start(out=mp[:, :], in_=map_d[:, :])
            nc.scalar.dma_start(out=st[:, :, :], in_=ts_d.ap()[:, :, :])

            # --- const generation on GpSimd (runs in the DMA shadow) ---
            itp = pool.tile([128, 1], I32)
            nc.gpsimd.iota(itp[:, :], [[0, 1]], channel_multiplier=1)
            itk = pool.tile([128, KLO], I32)
            nc.gpsimd.iota(itk[:, :], [[1, KLO]], channel_multiplier=0)
            flatidx = pool.tile([MAPP, MAPF], F32)
            nc.gpsimd.iota(flatidx[:, :], [[1, MAPF]], channel_multiplier=MAPF,
                           allow_small_or_imprecise_dtypes=True)

            jli = pool.tile([128, 1], I32)
            nc.vector.tensor_scalar(jli, itp, 3, None, op0=OP.arith_shift_right)
            kbi = pool.tile([128, 1], I32)
            nc.vector.tensor_scalar(kbi, itp, 7, None, op0=OP.bitwise_and)
            nc.vector.tensor_scalar(kbi, kbi, 4, None, op0=OP.logical_shift_left)

            jlf = pool.tile([128, 1], F32)
            nc.gpsimd.tensor_copy(jlf, jli)
            kbf = pool.tile([128, 1], F32)
            nc.gpsimd.tensor_copy(kbf, kbi)
            ktf = pool.tile([128, KLO], F32)
            nc.gpsimd.tensor_copy(ktf, itk)
            kvf = pool.tile([128, KLO], F32)
            nc.gpsimd.tensor_scalar(kvf, ktf, kbf[:, 0:1], None, op0=OP.add)

            # C weight matrix [p, klo, 4] = [1, jl, k, jl^2+k^2]
            c32 = pool.tile([128, KLO, 4], F32)
            nc.gpsimd.memset(c32[:, :, 0:1], 1.0)
            nc.gpsimd.tensor_scalar(c32[:, :, 1:2], c32[:, :, 0:1],
                                    jlf[:, 0:1], None, op0=OP.mult)
            nc.gpsimd.tensor_copy(c32[:, :, 2:3], kvf[:, :])
            jjf = pool.tile([128, 1], F32)
            nc.gpsimd.tensor_tensor(jjf, jlf, jlf, op=OP.mult)
            kkf = pool.tile([128, KLO], F32)
            nc.gpsimd.tensor_tensor(kkf, kvf, kvf, op=OP.mult)
            nc.gpsimd.tensor_scalar(c32[:, :, 3:4], kkf[:, :],
                                    jjf[:, 0:1], None, op0=OP.add)
            cw = pool.tile([128, KLO, 4], F32R)
            nc.gpsimd.tensor_copy(cw[:, :, :], c32[:, :, :])

            # --- argmax scalars (gated only by the map DMA) ---
            # Cross-partition data movement uses DVE 32x32 stream transposes
            # (gpsimd ucode ops like partition_all_reduce need a library
            # swap whose DMA starves behind the 1 MB input stream: ~7 us).
            sm = mp[:, 0:MAPF]
            scrA = pool.tile([MAPP, MAPP], F32)
            nc.vector.memset(scrA[:, :], 0.0)
            scrB = pool.tile([MAPP, MAPP], F32)
            nc.vector.memset(scrB[:, :], 0.0)

            # col 0 of scrA: per-partition max; col 0 of scrB: flat index of
            # each partition's own first max.
            nc.vector.tensor_reduce(scrA[:, 0:1], sm, axis=AX.X, op=OP.max)
            onehot = pool.tile([MAPP, MAPF], F32)
            nc.vector.scalar_tensor_tensor(
                onehot, in0=sm, scalar=scrA[:, 0:1], in1=flatidx,
                op0=OP.is_equal, op1=OP.mult, accum_out=scrB[:, 0:1])

            trA = pool.tile([MAPP, MAPP], F32)
            nc.vector.transpose(trA[:, :], scrA[:, :])
            trB = pool.tile([MAPP, MAPP], F32)
            nc.vector.transpose(trB[:, :], scrB[:, :])

            gmax = pool.tile([1, 1], F32)
            nc.vector.tensor_reduce(gmax, trA[0:1, :], axis=AX.X, op=OP.max)
            dum2 = pool.tile([1, MAPP], F32)
            flat = pool.tile([1, 1], F32)
            nc.vector.scalar_tensor_tensor(
                dum2, in0=trA[0:1, :], scalar=gmax[:, 0:1], in1=trB[0:1, :],
                op0=OP.is_equal, op1=OP.mult, accum_out=flat[:, 0:1])

            flti = pool.tile([1, 1], I32)
            nc.vector.tensor_copy(flti, flat)
            y0i = pool.tile([1, 1], I32)
            nc.vector.tensor_scalar(y0i, flti, 127, None, op0=OP.bitwise_and)
            x0i = pool.tile([1, 1], I32)
            nc.vector.tensor_scalar(x0i, flti, 7, None,
                                    op0=OP.logical_shift_right)
            y0f = pool.tile([1, 1], F32)
            nc.vector.tensor_copy(y0f, y0i)
            x0f = pool.tile([1, 1], F32)
            nc.vector.tensor_copy(x0f, x0i)
            x0p = pool.tile([1, 1], F32)   # x0' = x0 - jlo
            nc.vector.tensor_tensor(x0p, x0f, mp[0:1, MAPF:MAPF + 1],
                                    op=OP.subtract)

            # q row = [x0'^2 + y0^2, -2*x0', -2*y0, 1] in row 0 of a 32x32
            # scratch; one more stream transpose turns it into the [4,1]
            # column the final PE matmul wants.
            qsc = pool.tile([MAPP, MAPP], F32)
            nc.vector.memset(qsc[:, :], 0.0)
            nc.vector.memset(qsc[0:1, 3:4], 1.0)
            nc.vector.tensor_scalar(qsc[0:1, 1:2], x0p, -2.0, None, op0=OP.mult)
            nc.vector.tensor_scalar(qsc[0:1, 2:3], y0f, -2.0, None, op0=OP.mult)
            xx = pool.tile([1, 1], F32)
            nc.vector.tensor_tensor(xx, x0p, x0p, op=OP.mult)
            nc.vector.scalar_tensor_tensor(
                qsc[0:1, 0:1], in0=y0f, scalar=y0f[:, 0:1], in1=xx,
                op0=OP.mult, op1=OP.add)
            trQ = pool.tile([MAPP, MAPP], F32)
            nc.vector.transpose(trQ[:, :], qsc[:, :])
            qcol = trQ[0:4, 0:1]

            # --- main reduction: PSUM[4, w] += C[:,klo,:].T @ st[:,klo,:] ---
            psr = psum_pool.tile([4, W], F32)
            for klo in range(KLO):
                nc.tensor.matmul(psr[:, :], cw[:, klo, :], st[:, klo, :],
                                 start=(klo == 0), stop=(klo == KLO - 1))

            # --- combine: out = qcol.T @ [R0;R1;R2;R3] on the PE ---
            r4 = pool.tile([4, W], F32)
            nc.vector.tensor_copy(r4, psr[:, :])
            outp = psum_pool.tile([1, W], F32)
            nc.tensor.matmul(outp[:, :], qcol, r4[:, :],
                             start=True, stop=True)
            outv = pool.tile([1, W], F32)
            nc.vector.tensor_copy(outv, outp[:, :])

            nc.sync.dma_start(out=outd[:, :], in_=outv[:, :])

    return nc


def _get_bass():
    if "nc" not in _CACHE:
        nc = _build_bass()
        nc.finalize()
        _CACHE["nc"] = nc
    return _CACHE["nc"]


def _make_in_maps(tensor):
    t0 = np.ascontiguousarray(tensor[0], dtype=np.float32)  # [C,H,W]
    mp0 = t0[0].reshape(MAPP, MAPF)
    in_maps = []
    for c in range(NCORES):
        jlo = c * JPER
        mapx = np.empty((MAPP, MAPF + 1), dtype=np.float32)
        mapx[:, :MAPF] = mp0
        mapx[:, MAPF] = float(jlo)
        in_maps.append({
            "map": mapx,
            "tslice": np.ascontiguousarray(
                t0[jlo:jlo + JPER].reshape(128, KLO, W)),
        })
    return in_maps


def kernel(tensor):
    nc = _get_bass()
    res = run_bass_kernel_spmd(nc, _make_in_maps(tensor),
                               core_ids=list(range(NCORES)))
    partials = np.stack([r["out"].reshape(W) for r in res.results])
    return partials.astype(np.float64).sum(axis=0).astype(np.float32)


# revision 11
# speedup vs baseline: 1.4164x; 1.0084x over previous
"""Trainium2 Bass kernel for nn_Loss_39341900431615.

Reference semantics (B,C,H,W = 16,128,128,128; only tensor[0] is read):
    idx = argmax(tensor[0,0].reshape(-1))        # row-major first max
    x0, y0 = idx // W, idx % W
    wgt[j,k] = (x0-j)^2 + (y0-k)^2               # [H,W] = [128,128]
    out[w] = sum_{j,k} wgt[j,k] * tensor[0,j,k,w]  # [W] = [128]

Sharding: j (channel dim of tensor[0]) is split across 8 cores, 16
j-planes each (1 MB/core). Each core redundantly computes the argmax
from a replicated copy of tensor[0,0] and emits a [128] partial; the
host sums the 8 partials.

Key restructure vs the v1 kernel (which serialized argmax -> wgt ->
reduction): the weight factors as
    wgt[p,klo] = q0*1 + q1*jl(p) + q2*k(p,klo) + (jl(p)^2 + k(p,klo)^2)
with q0 = x0'^2+y0^2, q1 = -2*x0', q2 = -2*y0, x0' = x0 - jlo, and
jl(p) = p//8 the core-local j. So the big reduction is FOUR fixed-weight
sums R_i[w] = sum_{p,klo} C_i[p,klo]*st[p,klo,w] that do not depend on
the argmax at all: they run as PE matmuls (stationary C [128,4] f32r,
moving st [128,128] f32r, accumulating PSUM [4,128]) as soon as the
data lands. The argmax chain only has to produce three scalars in time
for a 3-op DVE combine at the very end:
    out[w] = q0*R0 + q1*R1 + q2*R2 + R3.

DMA plan (the v1 bottleneck was a [128 x 1096B] const-blob DMA whose
128 per-partition descriptors took ~2.5us to complete, plus consts
gated the whole chain):
  - no const blob at all: iota/shift/mask ops on GpSimd generate the
    C matrix, flat-index rows, etc. during the DMA shadow. The only
    per-core varying scalar (jlo) rides as one extra column of the map.
  - map+meta [32, 513] f32: 32 contiguous ~2KB descriptors (fast path).
  - tslice [128, 16, 128] f32r in TWO half-DMAs (klo 0..7, 8..15) on
    the ACT ring: first-half matmuls start ~1.7us before the tail of
    the transfer arrives.
  - fp32r matmuls: single-pass fp32 on the PE (tolerance is 2e-2;
    measured error stays ~1e-6 regardless).

Argmax without transposes (order-free because the max is unique in the
reference's random data): gmax = all-reduce max (GpSimd XYZWC), flat =
sum((map==gmax)*flatidx) via one DVE STT with accum_out + a GpSimd
C-axis reduce; x0 = flat>>7, y0 = flat&127 in int32; broadcastless
combine because the three q scalars and the PSUM result rows all live
on partitions 0..3 as [1,128]/[1,1] APs.

Framework facts this code is shaped by (measured on this machine):
  - walrus allows ONE sync wait per compute instruction; Bacc's
    generate_event_semaphores/move_matmul_waits_to_ldweights legalize
    multi-wait instructions, raw bass.Bass does not -> use bacc.Bacc
    and call nc.finalize() before compiling/running.
  - Bacc DCE removes dead instructions WITH their semaphore waits --
    never park a DMA wait on an instruction whose output nobody reads.
  - NRT adds ~11 us of fixed per-execution overhead (entry barrier +
    engine TENSOR_LOADs at ~3.4-4.9us + Tile preamble barrier to
    ~7.2us + full semaphore-space sweep at exit ~3.4us).
"""

import sys

for _p in ("/opt/trn_rl_repo", "/opt/pypackages"):
    if _p not in sys.path:
        sys.path.insert(0, _p)

import numpy as np

import concourse.bass as bass
from concourse import bacc
import concourse.tile as tile
from concourse import mybir
from concourse import bass_isa
from concourse.bass_utils import run_bass_kernel_spmd

B, C, H, W = 16, 128, 128, 128
NCORES = 8
JPER = C // NCORES   # 16 j-planes per core
KLO = 16             # contraction steps per partition (k within block)
KHI = 8              # k blocks per partition dim
MAPP = 32            # partitions of the contiguous map load
MAPF = (H * W) // MAPP  # 512 map elems per partition
PREBARRIER = True    # hoist input DMAs ahead of the entry barrier

F32 = mybir.dt.float32
F32R = mybir.dt.float32r
I32 = mybir.dt.int32
AX = mybir.AxisListType
OP = mybir.AluOpType

_CACHE = {}


def _build_bass():
    nonlocal_dmas = [None, None]
    nc = bacc.Bacc("TRN2", target_bir_lowering=False, debug=False,
                   num_devices=NCORES, enable_partition_id=False)

    # map+meta: cols 0..511 = tensor[0,0] row-major; col 512 row 0 = jlo
    map_d = nc.dram_tensor("map", [MAPP, MAPF + 1], F32, kind="ExternalInput")
    cm_d = nc.dram_tensor("cmat", [128, KLO * 4], F32R, kind="ExternalInput")
    ts_d = nc.dram_tensor("tslice", [128, KLO, W], F32R, kind="ExternalInput")
    outd = nc.dram_tensor("out", [1, W], F32, kind="ExternalOutput")

    with tile.TileContext(nc) as tc:
        with (
            tc.tile_pool(name="main", bufs=1) as pool,
            tc.tile_pool(name="psum", bufs=1, space="PSUM") as psum_pool,
        ):
            mp = pool.tile([MAPP, MAPF + 1], F32)
            cw = pool.tile([128, KLO, 4], F32R)
            st = pool.tile([128, KLO, W], F32R)

            # --- input DMAs (map first: it gates the scalar chain). The
            # emitted instruction objects are captured so they can be
            # hoisted ahead of the Tile entry barrier below.
            ents = nc.main_func.blocks[0].instructions
            _a = len(ents)
            nc.sync.dma_start(out=mp[:, :], in_=map_d[:, :])
            nc.sync.dma_start(
                out=cw[:, :, :],
                in_=cm_d.ap().rearrange("p (a b) -> p a b", a=KLO))
            sync_dmas = list(ents[_a:])
            _a = len(ents)
            nc.scalar.dma_start(out=st[:, :, :], in_=ts_d.ap()[:, :, :])
            scal_dmas = list(ents[_a:])

            # --- flat-index rows for the argmax (GpSimd, DMA shadow) ---
            flatidx = pool.tile([MAPP, MAPF], F32)
            nc.gpsimd.iota(flatidx[:, :], [[1, MAPF]], channel_multiplier=MAPF,
                           allow_small_or_imprecise_dtypes=True)

            # --- argmax scalars (gated only by the map DMA) ---
            # Cross-partition data movement uses DVE 32x32 stream transposes
            # (gpsimd ucode ops like partition_all_reduce need a library
            # swap whose DMA starves behind the 1 MB input stream: ~7 us).
            sm = mp[:, 0:MAPF]
            scrA = pool.tile([MAPP, MAPP], F32)
            nc.vector.memset(scrA[:, :], 0.0)
            scrB = pool.tile([MAPP, MAPP], F32)
            nc.vector.memset(scrB[:, :], 0.0)

            # col 0 of scrA: per-partition max; col 0 of scrB: flat index of
            # each partition's own first max.
            nc.vector.tensor_reduce(scrA[:, 0:1], sm, axis=AX.X, op=OP.max)
            onehot = pool.tile([MAPP, MAPF], F32)
            nc.vector.scalar_tensor_tensor(
                onehot, in0=sm, scalar=scrA[:, 0:1], in1=flatidx,
                op0=OP.is_equal, op1=OP.mult, accum_out=scrB[:, 0:1])

            trA = pool.tile([MAPP, MAPP], F32)
            nc.vector.transpose(trA[:, :], scrA[:, :])
            trB = pool.tile([MAPP, MAPP], F32)
            nc.vector.transpose(trB[:, :], scrB[:, :])

            gmax = pool.tile([1, 1], F32)
            nc.vector.tensor_reduce(gmax, trA[0:1, :], axis=AX.X, op=OP.max)
            dum2 = pool.tile([1, MAPP], F32)
            flat = pool.tile([1, 1], F32)
            nc.vector.scalar_tensor_tensor(
                dum2, in0=trA[0:1, :], scalar=gmax[:, 0:1], in1=trB[0:1, :],
                op0=OP.is_equal, op1=OP.mult, accum_out=flat[:, 0:1])

            flti = pool.tile([1, 1], I32)
            nc.vector.tensor_copy(flti, flat)
            y0i = pool.tile([1, 1], I32)
            nc.vector.tensor_scalar(y0i, flti, 127, None, op0=OP.bitwise_and)
            x0i = pool.tile([1, 1], I32)
            nc.vector.tensor_scalar(x0i, flti, 7, None,
                                    op0=OP.logical_shift_right)
            y0f = pool.tile([1, 1], F32)
            nc.vector.tensor_copy(y0f, y0i)
            x0f = pool.tile([1, 1], F32)
            nc.vector.tensor_copy(x0f, x0i)
            x0p = pool.tile([1, 1], F32)   # x0' = x0 - jlo
            nc.vector.tensor_tensor(x0p, x0f, mp[0:1, MAPF:MAPF + 1],
                                    op=OP.subtract)

            # q row = [x0'^2 + y0^2, -2*x0', -2*y0, 1] in row 0 of a 32x32
            # scratch; one more stream transpose turns it into the [4,1]
            # column the final PE matmul wants.
            qsc = pool.tile([MAPP, MAPP], F32)
            nc.vector.memset(qsc[:, :], 0.0)
            nc.vector.memset(qsc[0:1, 3:4], 1.0)
            nc.vector.tensor_scalar(qsc[0:1, 1:2], x0p, -2.0, None, op0=OP.mult)
            nc.vector.tensor_scalar(qsc[0:1, 2:3], y0f, -2.0, None, op0=OP.mult)
            xx = pool.tile([1, 1], F32)
            nc.vector.tensor_tensor(xx, x0p, x0p, op=OP.mult)
            nc.vector.scalar_tensor_tensor(
                qsc[0:1, 0:1], in0=y0f, scalar=y0f[:, 0:1], in1=xx,
                op0=OP.mult, op1=OP.add)
            trQ = pool.tile([MAPP, MAPP], F32)
            nc.vector.transpose(trQ[:, :], qsc[:, :])
            qcolr = pool.tile([4, 1], F32R)
            nc.vector.tensor_copy(qcolr, trQ[0:4, 0:1])

            # --- main reduction: PSUM[4, w] += C[:,klo,:].T @ st[:,klo,:] ---
            psr = psum_pool.tile([4, W], F32)
            for klo in range(KLO):
                nc.tensor.matmul(psr[:, :], cw[:, klo, :], st[:, klo, :],
                                 start=(klo == 0), stop=(klo == KLO - 1))

            # --- combine: out = qcol.T @ [R0;R1;R2;R3] on the PE.
            # PSUM->SBUF copies ride the otherwise idle GpSimd so the DVE
            # argmax chain never blocks the tail.
            r4 = pool.tile([4, W], F32R)
            nc.scalar.activation(r4, psr[:, :],
                                 func=mybir.ActivationFunctionType.Copy)
            outp = psum_pool.tile([1, W], F32)
            nc.tensor.matmul(outp[:, :], qcolr[:, :], r4[:, :],
                             start=True, stop=True)
            outv = pool.tile([1, W], F32)
            nc.scalar.activation(outv, outp[:, :],
                                 func=mybir.ActivationFunctionType.Copy)

            nc.sync.dma_start(out=outd[:, :], in_=outv[:, :])

            nonlocal_dmas[0] = sync_dmas
            nonlocal_dmas[1] = scal_dmas

    sync_dmas_g, scal_dmas_g = nonlocal_dmas
    if PREBARRIER:
        # Hoist the input DMAs ahead of the Tile entry barrier: they only
        # read ExternalInput DRAM (valid from launch) and write SBUF tiles
        # nothing in the preamble touches, and the semaphore range-clear
        # runs at EXIT, so completion increments are never wiped. Saves
        # ~1.6us of dead time before the first descriptor hits the queue.
        entry = nc.main_func.blocks[0]
        for objs, eng in ((sync_dmas_g, nc.sync), (scal_dmas_g, nc.scalar)):
            for o in reversed(objs):
                entry.instructions.remove(o)
                idx = entry.instructions.index(eng.preamble_end) + 1
                entry.instructions.insert(idx, o)
    return nc


def _get_bass():
    if "nc" not in _CACHE:
        nc = _build_bass()
        nc.finalize()
        _CACHE["nc"] = nc
    return _CACHE["nc"]


def _host_cmat():
    if "cmat" not in _CACHE:
        p = np.arange(128)
        jl = (p // KHI).astype(np.float32)
        kv = ((p % KHI) * KLO)[:, None] + np.arange(KLO)[None, :]
        kv = kv.astype(np.float32)
        cm = np.empty((128, KLO, 4), dtype=np.float32)
        cm[:, :, 0] = 1.0
        cm[:, :, 1] = jl[:, None]
        cm[:, :, 2] = kv
        cm[:, :, 3] = (jl * jl)[:, None] + kv * kv
        _CACHE["cmat"] = np.ascontiguousarray(cm.reshape(128, KLO * 4))
    return _CACHE["cmat"]


def _make_in_maps(tensor):
    t0 = np.ascontiguousarray(tensor[0], dtype=np.float32)  # [C,H,W]
    mp0 = t0[0].reshape(MAPP, MAPF)
    cmat = _host_cmat()
    in_maps = []
    for c in range(NCORES):
        jlo = c * JPER
        mapx = np.empty((MAPP, MAPF + 1), dtype=np.float32)
        mapx[:, :MAPF] = mp0
        mapx[:, MAPF] = float(jlo)
        in_maps.append({
            "map": mapx,
            "cmat": cmat,
            "tslice": np.ascontiguousarray(
                t0[jlo:jlo + JPER].reshape(128, KLO, W)),
        })
    return in_maps


def kernel(tensor):
    nc = _get_bass()
    res = run_bass_kernel_spmd(nc, _make_in_maps(tensor),
                               core_ids=list(range(NCORES)))
    partials = np.stack([r["out"].reshape(W) for r in res.results])
    return partials.astype(np.float64).sum(axis=0).astype(np.float32)
